# revision 32
# baseline (speedup 1.0000x reference)
"""HD95 loss kernel for Trainium2 (Bass/Tile), 8 NeuronCores.

Reference semantics: per image, threshold pred/true at 0.5, compact nonzero
pixel indices in row-major order, split each point list into blocks of 1000,
and for every (point, opposite-side block) pair take the min Euclidean
distance; the HD95 is the 95th linear-interpolation quantile over all finite
such mins (both directions), averaged over the batch.

Device algorithm (per image & direction, "queries" vs "ref blocks"):
separable squared-EDT in two matmul stages.

  stage 1:  g[x, (blk, c)] = min over pixels (a) of image-row b0+c within
            block blk of (x-a)^2, via a contraction-5 bf16 matmul
            ([x2h,x2l,x,1,1] x [1,1,-2a,a2h,a2l]) over P=64 packed pixel
            slots per candidate row (sentinel [0,0,0,2^26,0] in pad slots),
            then a min-reduce per 64-slot group.  g is kept in bf16
            (rel err <= 2^-9, well inside the 2e-2 gate).
  stage 2:  min d^2(q, blk) = min_c ( (y_q - (b0+c))^2 + g[x_q, c] )
            one accumulating bf16 matmul per 128-query tile:
            [onehot(x_q); yfeat(y_q)] @ [g; rtop], then a min-reduce over
            the CAND candidates of each block.

The onehot(x_q) half of the stage-2 lhsT (96x2560, ~500KB as bf16) is
derived on device from an uploaded uint8 |x_q - p| matrix (245KB) with a
single Scalar act, relu(1 - d) — exact 0/1 for integer coords.  This
keeps the PE free of broadcast matmuls so stage 1 starts as soon as its
DMA lands, which matters because the Vector engine's min-reduce stream
(~1.04ns/elem, no fast modes — measured) is the critical path and starts
right behind the first stage-1 matmul.  Input DMA (~360KB) is split
across both HWDGE queues (SP + Activation), each ~26GB/s.  (GpSimd
tensor ops measured ~15ns/elem — never used for bulk work.)

Core mapping: 8 cores = 4 (image x direction) jobs x 2 halves of 2560
query slots. Host does the O(N) compaction/feature build and the final
O(50k) quantile; device does all O(K x window) distance work.
"""

import numpy as np

H = 96
W = 96
BLK = 1000        # reference cdist block size
NBLK = 5          # blocks per side (asserted from the data regime)
CAND = 24         # stage-2 candidate rows per block (spans <= 23 here)
# stage-1 candidate windows per block; block 4 holds only ~600 points
# spanning <= 14 rows, so its window is statically 16 (fallback if exceeded);
# its missing 8 g-columns are memset to the sentinel on device
CANDS = [24, 24, 24, 24, 16]
NROWS = sum(CANDS)  # 112 stage-1 rows
P = 64            # packed pixel slots per candidate row (max row count 64)
CHUNK = 8 * P     # stage-1 matmul free size (512: one PSUM bank, 8 rows)
S1GROUPS = [1, 2, 3, 3, 3, 2]  # chunks per reduce group (14 chunks); a tiny
                               # first group starts the Vector stream early
QHALF = 2560      # query slots per core (20 tiles of 128)
NTILES = QHALF // 128
S2GROUPS = [4, 4, 4, 4, 3, 1]  # stage-2 query tiles per psum group; small
                               # final groups shorten the closing chain
BIG = float(2 ** 26)  # sentinel (bf16-exact, >> max real d^2 of 18050)
NCORES = 8
S1COLS = 96 + NROWS * P  # 96 lhsT cols + 7168 slot cols

_CACHE = {}


def _build_nc():
    import concourse.bacc as bacc
    import concourse.mybir as mybir
    import concourse.tile as tile

    f32 = mybir.dt.float32
    bf16 = mybir.dt.bfloat16
    # Bacc (not raw Bass): its compile() runs move_matmul_waits_to_ldweights
    # + generate_event_semaphores, which legalize multi-wait instructions
    # (TRN2 allows at most one sync wait per instruction).
    nc = bacc.Bacc("TRN2", target_bir_lowering=False, debug=False)

    u8 = mybir.dt.uint8
    s1_pack = nc.declare_dram_parameter("s1_pack", [5, S1COLS], bf16, isOutput=False)
    absdiff = nc.declare_dram_parameter("absdiff", [96, QHALF], u8, isOutput=False)
    yfeat = nc.declare_dram_parameter("yfeat", [5, QHALF], bf16, isOutput=False)
    rtop = nc.declare_dram_parameter("rtop", [5, NBLK * CAND], bf16, isOutput=False)
    mins = nc.declare_dram_parameter("mins", [128, NTILES * NBLK], f32, isOutput=True)

    X = mybir.AxisListType.X
    MIN = mybir.AluOpType.min
    RELU = mybir.ActivationFunctionType.Relu

    with tile.TileContext(nc) as tc:
        with (
            tc.tile_pool(name="const", bufs=1) as const,
            tc.tile_pool(name="ps1", bufs=2, space="PSUM") as ps1,
            tc.tile_pool(name="ps2", bufs=2, space="PSUM") as ps2,
        ):
            t_s1 = const.tile([5, S1COLS], bf16)
            # rows 0..95: onehot(x_q) (device-built); rows 96..100: yfeat (DMA)
            t_lhsT2 = const.tile([101, QHALF], bf16)
            t_diff = const.tile([96, QHALF], u8)
            # rows 0..95: g (bf16, reduce-written); rows 96..100: rtop (DMA)
            t_ghr = const.tile([101, NBLK * CAND], bf16)
            t_out = const.tile([128, NTILES * NBLK], f32)
            t_s1_lhsT = t_s1[:, 0:96]

            # DMA plan: the Vector-critical stage-1 stream owns the Sync
            # queue — a small first piece so the first matmul fires ~9.5us,
            # then progressively larger pieces that land just ahead of their
            # matmul groups.  The bulky uint8 |x_q - p| matrix rides last on
            # both queues: the onehot relu only has to beat the stage-2
            # start (~18us), which leaves it ~4us of slack.
            c1, c2 = 96 + 3 * CHUNK, 96 + 9 * CHUNK
            nc.sync.dma_start(t_s1[:, 0:c1], s1_pack[:, 0:c1])
            nc.sync.dma_start(t_s1[:, c1:c2], s1_pack[:, c1:c2])
            nc.sync.dma_start(t_s1[:, c2:], s1_pack[:, c2:])
            nc.scalar.dma_start(t_lhsT2[96:101, :], yfeat[:])
            nc.scalar.dma_start(t_ghr[96:101, :], rtop[:])
            nc.scalar.dma_start(t_diff[0:48, :], absdiff[0:48, :])
            nc.sync.dma_start(t_diff[48:96, :], absdiff[48:96, :])

            # block 4's stage-1 window is 16 rows; its unused 8 g-columns
            # stay at the sentinel so stage 2's uniform 24-cand min works
            nc.gpsimd.memset(t_ghr[0:96, NROWS : NBLK * CAND], BIG)

            # onehot(x_q) = relu(1 - |x_q - p|), exact 0/1 for integer
            # coords, one act on the otherwise-idle Scalar engine
            nc.scalar.activation(
                t_lhsT2[0:96, :], t_diff[:], RELU, bias=1.0, scale=-1.0
            )

            # stage 1: g[x, (blk, cand)] -> t_ghr rows 0..95 (bf16)
            ck = 0
            for ng in S1GROUPS:
                ps = ps1.tile([96, 3, 512], f32, tag="ps1")
                for k in range(ng):
                    c0 = 96 + (ck + k) * CHUNK
                    nc.tensor.matmul(
                        ps[:, k, :], t_s1_lhsT, t_s1[:, c0 : c0 + CHUNK],
                        start=True, stop=True,
                    )
                r0 = ck * (CHUNK // P)
                red_out = t_ghr[0:96, r0 : r0 + ng * (CHUNK // P)].rearrange(
                    "p (c u) -> p c u", c=ng
                )
                red_in = ps[:, 0:ng, :].rearrange("p c (u a) -> p c u a", a=P)
                nc.vector.tensor_reduce(red_out, red_in, axis=X, op=MIN)
                ck += ng

            # stage 2: one matmul per 128-query tile, then min over candidates
            t0 = 0
            for gsz in S2GROUPS:
                ps_o = ps2.tile([128, gsz, NBLK, CAND], f32, tag="ps2")
                for j in range(gsz):
                    t = t0 + j
                    ts = slice(t * 128, (t + 1) * 128)
                    nc.tensor.matmul(
                        ps_o[:, j, :, :], t_lhsT2[:, ts], t_ghr[:],
                        start=True, stop=True,
                    )
                nc.vector.tensor_reduce(
                    t_out[:, t0 * NBLK : (t0 + gsz) * NBLK].rearrange(
                        "p (j b) -> p j b", j=gsz
                    ),
                    ps_o[:, :, :, :], axis=X, op=MIN,
                )
                t0 += gsz

            # output in two pieces so the first 16 tiles' mins stream out
            # while the final small groups finish; the second piece issues
            # from the (idle) Scalar queue in parallel with the first
            c0 = 16 * NBLK
            nc.sync.dma_start(mins[:, 0:c0], t_out[:, 0:c0])
            nc.scalar.dma_start(mins[:, c0:], t_out[:, c0:])

    nc.compile()
    return nc


def _get_nc():
    if "nc" not in _CACHE:
        _CACHE["nc"] = _build_nc()
    return _CACHE["nc"]


def _bf16(a):
    from ml_dtypes import bfloat16

    return np.asarray(a, np.float32).astype(bfloat16)


def _hilo(v):
    """Split integer-valued array into (multiple-of-128, remainder<128)."""
    v = np.asarray(v, np.float64)
    lo = np.mod(v, 128.0)
    return (v - lo).astype(np.float32), lo.astype(np.float32)


def _side_points(img):
    """Compacted nonzero pixel coords, row-major ascending (matches
    jnp.nonzero order)."""
    m = (np.asarray(img) > 0.5).reshape(-1)
    idx = np.nonzero(m)[0]
    ys = (idx // W).astype(np.int64)
    xs = (idx % W).astype(np.int64)
    return ys, xs


def _feat5_queries(vals):
    """[v2h, v2l, v, 1, 1] feature rows for the squared-term side."""
    v = np.asarray(vals, np.float64)
    h, l = _hilo(v * v)
    one = np.ones_like(v, np.float32)
    return np.stack([h, l, v.astype(np.float32), one, one])


def _feat5_refs(vals):
    """[1, 1, -2v, v2h, v2l] feature rows for the reference side."""
    v = np.asarray(vals, np.float64)
    h, l = _hilo(v * v)
    one = np.ones_like(v, np.float32)
    return np.stack([one, one, (-2.0 * v).astype(np.float32), h, l])


def _build_core_inputs(q_ys, q_xs, r_ys, r_xs):
    """Host-side feature build for one (image, direction) job.

    q_*: query points (cnt_q), r_*: reference points (cnt_r, split into
    NBLK blocks of BLK in compacted order). Returns two per-core input
    maps, or None if the data falls outside the compiled regime.
    """
    cnt_q, cnt_r = len(q_ys), len(r_ys)
    if not (0 < cnt_q <= 2 * QHALF and 0 < cnt_r <= NBLK * BLK):
        return None
    if (cnt_r + BLK - 1) // BLK != NBLK:
        return None

    s1_lhsT = _feat5_queries(np.arange(96))

    # packed slots: pixel j of a candidate row sits at slot j; per-block
    # stage-1 windows CANDS (block 4's tail rows are device-memset sentinel)
    s1_rhs = np.zeros((5, NROWS, P), np.float32)
    s1_rhs[3] = BIG  # sentinel [0, 0, 0, BIG, 0]
    s2_rtop = np.empty((5, NBLK, CAND), np.float32)
    row0 = 0
    for blk in range(NBLK):
        lo, hi = blk * BLK, min((blk + 1) * BLK, cnt_r)
        ys_b, xs_b = r_ys[lo:hi], r_xs[lo:hi]
        b0 = int(ys_b[0])
        cand_b = CANDS[blk]
        if int(ys_b[-1]) - b0 + 1 > cand_b:
            return None
        # row-major order: pixels of one row are contiguous in ys_b
        starts = np.searchsorted(ys_b, b0 + np.arange(cand_b + 1))
        if (starts[1:] - starts[:-1]).max() > P:
            return None
        for c in range(cand_b):
            s, e = starts[c], starts[c + 1]
            if e > s:
                s1_rhs[:, row0 + c, 0 : e - s] = _feat5_refs(xs_b[s:e])
        s2_rtop[:, blk, :] = _feat5_refs(b0 + np.arange(CAND))
        row0 += cand_b
    s1_pack = _bf16(np.concatenate([s1_lhsT, s1_rhs.reshape(5, -1)], axis=1))

    xq_full = np.zeros(2 * QHALF, np.int64)
    xq_full[:cnt_q] = q_xs
    # |x_q - p| for the device-side relu(1 - d) onehot build
    absd = np.abs(
        xq_full[None, :] - np.arange(96, dtype=np.int64)[:, None]
    ).astype(np.uint8)
    yf_full = np.zeros((5, 2 * QHALF), np.float32)
    yf_full[:, :cnt_q] = _feat5_queries(q_ys)

    maps = []
    for half in range(2):
        hs = slice(half * QHALF, (half + 1) * QHALF)
        maps.append(
            {
                "s1_pack": s1_pack,
                "absdiff": absd[:, hs],
                "yfeat": _bf16(yf_full[:, hs]),
                "rtop": _bf16(s2_rtop.reshape(5, -1)),
            }
        )
    return maps


def _quantile95(vals):
    """torch.quantile / jnp.nanquantile 'linear' on finite values."""
    v = np.sort(np.asarray(vals, np.float64))
    n = v.size
    pos = 0.95 * (n - 1)
    lo = int(np.floor(pos))
    hi = min(lo + 1, n - 1)
    frac = pos - lo
    return v[lo] * (1.0 - frac) + v[hi] * frac


def _hd95_numpy_fallback(pred, true):
    """Pure-numpy path for data outside the compiled regime."""
    p_ys, p_xs = _side_points(pred)
    t_ys, t_xs = _side_points(true)
    if len(p_ys) == 0 or len(t_ys) == 0:
        return None
    pc = np.stack([p_ys, p_xs], -1).astype(np.float32)
    tc = np.stack([t_ys, t_xs], -1).astype(np.float32)
    vals = []
    for qc, rc in ((pc, tc), (tc, pc)):
        nbr = (len(rc) + BLK - 1) // BLK
        for jb in range(nbr):
            b = rc[jb * BLK : (jb + 1) * BLK]
            d2 = (
                (qc * qc).sum(-1)[:, None]
                + (b * b).sum(-1)[None, :]
                - 2.0 * (qc @ b.T)
            )
            vals.append(np.sqrt(np.maximum(d2.min(1), 0.0).astype(np.float32)))
    return _quantile95(np.concatenate(vals))


def _run_device(in_maps, trace=False):
    from concourse.bass_utils import run_bass_kernel_spmd

    nc = _get_nc()
    return run_bass_kernel_spmd(nc, in_maps, list(range(NCORES)), trace=trace)


def kernel(input, target, _trace=False, _results_out=None):
    input = np.asarray(input)
    target = np.asarray(target)
    nimg = input.shape[0]

    # jobs: (image, direction). dir 0: queries=pred, refs=true (row mins);
    # dir 1: queries=true, refs=pred (col mins).
    jobs = []
    in_maps = []
    fallback = {}
    ok_mask = []
    for i in range(nimg):
        p_ys, p_xs = _side_points(input[i])
        t_ys, t_xs = _side_points(target[i])
        ok = len(p_ys) > 0 and len(t_ys) > 0
        ok_mask.append(ok)
        if not ok:
            continue
        built_row = _build_core_inputs(p_ys, p_xs, t_ys, t_xs)
        built_col = _build_core_inputs(t_ys, t_xs, p_ys, p_xs)
        if built_row is None or built_col is None or nimg != 2:
            fallback[i] = _hd95_numpy_fallback(input[i], target[i])
            continue
        jobs.append((i, 0, len(p_ys)))
        in_maps.extend(built_row)
        jobs.append((i, 1, len(t_ys)))
        in_maps.extend(built_col)

    hds = {}
    if jobs:
        while len(in_maps) < NCORES:  # pad to the full 8-core SPMD launch
            in_maps.append({k: v.copy() for k, v in in_maps[0].items()})
        try:
            res = _run_device(in_maps[:NCORES], trace=_trace)
            if _results_out is not None:
                _results_out.append(res)
            per_img_vals = {}
            for j, (img, _dir, cnt_q) in enumerate(jobs):
                # out col = tile*NBLK + b, query tiles in order (S2GROUPS
                # layout is sequential in tile index)
                halves = []
                for o in (res.results[2 * j]["mins"], res.results[2 * j + 1]["mins"]):
                    o = o.reshape(128, NTILES, NBLK)
                    halves.append(o.transpose(1, 0, 2).reshape(QHALF, NBLK))
                d2 = np.concatenate(halves)[:cnt_q]
                assert d2.max() < 2.0 ** 25, "sentinel leaked into mins"
                dist = np.sqrt(d2.astype(np.float32))
                per_img_vals.setdefault(img, []).append(dist.ravel())
            for img, chunks in per_img_vals.items():
                hds[img] = _quantile95(np.concatenate(chunks))
        except Exception:
            # device path failed — stay correct via the numpy route
            for img, _dir, _cnt in jobs:
                if img not in fallback:
                    fallback[img] = _hd95_numpy_fallback(input[img], target[img])
    hds.update(fallback)

    n_ok = sum(ok_mask)
    if n_ok == 0:
        return np.float32(np.inf)
    total = sum(hds[i] for i in range(nimg) if ok_mask[i])
    return np.float32(total / n_ok)
